# revision 51
# baseline (speedup 1.0000x reference)
"""Causal multi-head attention (B=4, N=2048, D=768, H=12) on 8 TRN2 cores.

Sharding: batch (4) x head-split (2). Core c = (b, hg) handles batch b
and heads 6*hg .. 6*hg+5 over the FULL sequence: QKV projections take
only this half's weight columns, attention runs 6 heads, and the out
projection uses only this half's weight rows, producing a partial
[N, D] that the host sums across the pair of cores sharing a batch.

All matmul operands are bf16 (fp32 PSUM accumulation). Per core:
  prologue: qT/kT pair 0 = Wqk^T x^T, v = x Wv (+ones col, all pairs)
  per pair m: per (head, query-half, key-tile jt), software-pipelined
  (scores for jt+1 issue before AV jt), with pair m+1's q/k projection
  matmuls interleaved one-per-jt into the loop so the PE never idles
  (keeps the HAM clock gate at 8/8) and the projection phase costs no
  extra wall-clock:
    s = kT_jt^T qT;  et = exp(s/8) bf16 (* 0/1 causal mask on diag
    tiles, GPSIMD);  oT[65, :] += v_aug_jt^T et (row 64 = denominators)
  aT = oT[0:64] (pair-packed via DMA repartition for odd heads)
  rec broadcast via PE selector-matmul into PSUM (deferred into the
  next section's loop);  aTb = aT * rec bf16;  o_partial = aTb^T Wo
"""

import numpy as np

B, N, D, H = 4, 2048, 768, 12
DH = D // H          # 64
HH = H // 2          # 6 local heads per core
NPAIR = HH // 2      # 3 head pairs
KC = D // 128        # 6 contraction chunks
VW = HH * (DH + 1)   # 390 (v_aug row width per seq tile)
NEG = -30000.0

_CACHE = {}


def _build_nc(unused=None):
    import concourse.bacc as bacc
    import concourse.bass as bass  # noqa: F401
    import concourse.mybir as mybir
    import concourse.tile as tile
    from contextlib import ExitStack

    dt = mybir.dt
    f32 = dt.float32
    bf16 = dt.bfloat16
    Exp = mybir.ActivationFunctionType.Exp

    nc = bacc.Bacc(None)
    xt = nc.declare_dram_parameter("xt", [D, N], bf16, isOutput=False)
    wqk = nc.declare_dram_parameter("wqk", [D, 2 * HH * DH], bf16,
                                    isOutput=False)
    wv = nc.declare_dram_parameter("wv", [D, HH * DH], bf16, isOutput=False)
    wo = nc.declare_dram_parameter("wo", [HH * DH, D], bf16, isOutput=False)
    cst = nc.declare_dram_parameter("cst", [3, 128, 128], bf16, isOutput=False)
    o = nc.declare_dram_parameter("o", [N, D], bf16, isOutput=True)

    with tile.TileContext(nc) as tc:
        with ExitStack() as es:
            persist = es.enter_context(tc.tile_pool(name="persist", bufs=1))
            qT = [persist.tile([128, N], bf16, tag=f"qT{m}", name=f"qT{m}")
                  for m in range(NPAIR)]
            kT = [persist.tile([128, N], bf16, tag=f"kT{m}", name=f"kT{m}")
                  for m in range(NPAIR)]
            # +64 pad cols: AV lhsT slices extend to 128 weight columns
            # (rows 65..127 of oT are don't-care) so the PE array runs
            # full-width and the HAM activity monitor keeps the clock at 8/8
            vp_all = persist.tile([128, 16 * VW + 64], bf16, tag="vp",
                                  name="vp")
            vp = [vp_all[:, j * VW:(j + 1) * VW] for j in range(16)]
            aT = [persist.tile([128, N], f32, tag=f"aT{m}", name=f"aT{m}")
                  for m in range(NPAIR)]
            aTb = [persist.tile([128, N], bf16, tag=f"aTb{m}", name=f"aTb{m}")
                   for m in range(NPAIR)]
            # den rows live at partitions 0 (even head) and 64 (odd head) so
            # the PE rec-broadcast lhsT/rhs base partitions align.
            den = [persist.tile([65, N], f32, tag=f"den{m}", name=f"den{m}")
                   for m in range(NPAIR)]
            recb = persist.tile([128, N], bf16, tag="recb", name="recb")
            for m in range(NPAIR):
                nc.vector.memset(den[m][:], 1.0)
            nc.vector.memset(recb[:], 1.0)
            msk = persist.tile([128, 384], bf16, tag="msk", name="msk")
            tri01 = msk[:, 0:128]
            sel = msk[:, 128:256]
            ident = msk[:, 256:384]
            wos = persist.tile([128, NPAIR * D], bf16, tag="wos", name="wos")
            # pairs 0+1 output-projection partial, accumulated during
            # pair-2's attention (real PE work instead of a heater)
            oacc = persist.tile([128, 16 * D], bf16, tag="oacc", name="oacc")
            xts = persist.tile([128, KC * N], bf16, tag="xts", name="xts")
            wqks = persist.tile([128, KC * 768], bf16, tag="wqks",
                                name="wqks")
            wvs = persist.tile([128, KC * 384], bf16, tag="wvs", name="wvs")
            ones96 = persist.tile([128, 16 * HH], bf16, tag="on",
                                  name="ones96")

            etp = es.enter_context(tc.tile_pool(name="et", bufs=5))
            dtp = es.enter_context(tc.tile_pool(name="dtm", bufs=2))
            omp = es.enter_context(tc.tile_pool(name="omp", bufs=1))
            osb = es.enter_context(tc.tile_pool(name="osb", bufs=3))
            ps4 = es.enter_context(tc.tile_pool(name="ps4", bufs=2,
                                                space="PSUM"))
            po4 = es.enter_context(tc.tile_pool(name="po4", bufs=1,
                                                space="PSUM"))
            pj = es.enter_context(tc.tile_pool(name="pj", bufs=1,
                                               space="PSUM"))

            # ---------------- input DMAs ----------------
            # msk first: the PE warmup below reads it as soon as it lands
            for i2 in range(3):
                nc.sync.dma_start(out=msk[:, i2 * 128:(i2 + 1) * 128],
                                  in_=cst[i2])
            for k in range(KC):
                nc.scalar.dma_start(out=wqks[:, k * 768:(k + 1) * 768],
                                    in_=wqk[k * 128:(k + 1) * 128, :])
                nc.sync.dma_start(out=xts[:, k * N:k * N + 512],
                                  in_=xt[k * 128:(k + 1) * 128, 0:512])
            for k in range(KC):
                nc.gpsimd.dma_start(out=wvs[:, k * 384:(k + 1) * 384],
                                    in_=wv[k * 128:(k + 1) * 128, :])
            for qtr in range(1, 4):
                c0 = qtr * 512
                for k in range(KC):
                    nc.sync.dma_start(
                        out=xts[:, k * N + c0:k * N + c0 + 512],
                        in_=xt[k * 128:(k + 1) * 128, c0:c0 + 512])
            for f in range(NPAIR):
                nc.gpsimd.dma_start(out=wos[:, f * D:(f + 1) * D],
                                    in_=wo[f * 128:(f + 1) * 128, :])

            # ---------------- projection helpers ----------------
            def qkproj_gen(m):
                """q/k projection for pair m; yields once per matmul so the
                attention loop can interleave it one matmul per key tile."""
                for qtr in range(4):
                    c0 = qtr * 512
                    for half in range(2):
                        dstT = qT[m] if half == 0 else kT[m]
                        tg = "pjq" if half == 0 else "pjk"
                        ps = pj.tile([128, 512], f32, tag=tg, name=tg)
                        cb = 384 * half + m * 128
                        for k in range(KC):
                            nc.tensor.matmul(
                                out=ps[:],
                                lhsT=wqks[:, k * 768 + cb:
                                          k * 768 + cb + 128],
                                rhs=xts[:, k * N + c0:k * N + c0 + 512],
                                start=(k == 0), stop=(k == KC - 1),
                                skip_group_check=True)
                            if k == KC - 1:
                                nc.vector.tensor_copy(dstT[:, c0:c0 + 512],
                                                      ps[:])
                            yield

            def vproj_gen(st_lo, st_hi):
                for st in range(st_lo, st_hi):
                    tg = "pjq" if st % 2 == 0 else "pjk"
                    ps = pj.tile([128, 512], f32, tag=tg, name=tg)
                    for k in range(KC):
                        nc.tensor.matmul(
                            out=ps[:, 0:384],
                            lhsT=xts[:, k * N + st * 128:
                                     k * N + (st + 1) * 128],
                            rhs=wvs[:, k * 384:(k + 1) * 384],
                            start=(k == 0), stop=(k == KC - 1),
                            skip_group_check=True)
                        if k == KC - 1:
                            vv = vp[st].rearrange("p (h c) -> p h c",
                                                  c=DH + 1)
                            nc.vector.tensor_copy(
                                vv[:, :, 0:DH],
                                ps[:, 0:384].rearrange("p (h c) -> p h c",
                                                       c=DH))
                        yield

            # preload the exp activation-table set (~2.7us) during the DMA
            # ramp so pair-0's first real exp doesn't pay it (recb[0,0] is
            # rewritten by the first real recb copy before anything reads it)
            nc.scalar.activation(out=recb[0:1, 0:1], in_=recb[0:1, 0:1],
                                 func=Exp, scale=0.125)
            # ones column (disjoint from the vproj drains, so emitted early)
            nc.vector.memset(ones96[:], 1.0)
            nc.vector.memset(vp_all[:, 16 * VW:], 1.0)
            vview = vp_all[:, 0:16 * VW].rearrange("p (j c) -> p j c",
                                                   c=DH + 1)
            nc.vector.tensor_copy(
                vview[:, :, DH:DH + 1],
                ones96[:].rearrange("p (a b) -> p a b", b=1))

            # ---------------- prologue: pair-0 q/k + v st 0..11 ----------
            # Short PE warmup on the msk constant while the x/w DMAs land:
            # ~3.4us of dense activity flips the HAM clock gate to 8/8 so
            # the projection runs at 2.4GHz. Sized to end as the DMAs
            # arrive — a longer burst would block the projection (in-order
            # PE), which is why a 48-matmul warmup measured slower.
            wps = pj.tile([128, 512], f32, tag="pjq", name="pjq")
            for _ in range(20):
                nc.tensor.matmul(out=wps[:, 0:256], lhsT=msk[:, 0:128],
                                 rhs=msk[:, 0:256], start=True, stop=True,
                                 skip_group_check=True)
            # (v st 12..15 are interleaved into pair-0's first section)
            for _ in qkproj_gen(0):
                pass
            for _ in vproj_gen(0, 12):
                pass

            # ---------------- attention ----------------
            # normalize PE-work (rec broadcast + multiply) is deferred into
            # a later jt loop so the PE never waits on the DVE den->rec chain
            pnorm = []

            def flush_norm(pool):
                while pnorm:
                    m_, q0_ = pnorm.pop(0)
                    rbps = pool.tile([128, 1024], f32, tag="ps", name="ps")
                    for (a, b) in ((0, 512), (512, 1024)):
                        nc.tensor.matmul(
                            out=rbps[:, a:b], lhsT=sel,
                            rhs=recb[:, q0_ + a:q0_ + b],
                            start=True, stop=True,
                            skip_group_check=True)
                    nc.vector.tensor_mul(
                        aTb[m_][:, q0_:q0_ + 1024],
                        aT[m_][:, q0_:q0_ + 1024], rbps[:])

            def yp_gen():
                """Output projection f=0,1 partials -> oacc (bf16). Runs as
                pair-2's filler: keeps the PE (and the HAM clock gate) busy
                with work the tail would otherwise do."""
                for st in range(16):
                    pq = pj.tile([128, 512], f32, tag="pjq", name="pjq")
                    pk = pj.tile([128, 512], f32, tag="pjk", name="pjk")
                    for f in range(2):
                        nc.tensor.matmul(
                            out=pq[:],
                            lhsT=aTb[f][:, st * 128:(st + 1) * 128],
                            rhs=wos[:, f * D:f * D + 512],
                            start=(f == 0), stop=(f == 1),
                            skip_group_check=True)
                        yield
                        nc.tensor.matmul(
                            out=pk[:, 0:256],
                            lhsT=aTb[f][:, st * 128:(st + 1) * 128],
                            rhs=wos[:, f * D + 512:f * D + 768],
                            start=(f == 0), stop=(f == 1),
                            skip_group_check=True)
                        if f == 1:
                            nc.vector.tensor_copy(
                                oacc[:, st * D:st * D + 512], pq[:])
                            nc.vector.tensor_copy(
                                oacc[:, st * D + 512:(st + 1) * D],
                                pk[:, 0:256])
                        yield

            def emit_pair(m, filler_map):
                otmp = omp.tile([64, N], f32, tag="otmp", name="otmp")
                # qc=1 first: pair-0's first section then has 16 key tiles
                # of runway for the interleaved tail of the v projection
                for (hh, qc) in ((0, 1), (0, 0), (1, 0), (1, 1)):
                    if True:
                        h = 2 * m + hh
                        hs = slice(hh * 64, hh * 64 + 64)
                        q0 = qc * 1024
                        filler, pops = filler_map.get((hh, qc), (None, 0))
                        oT = po4.tile([128, 1024], f32, tag="oT", name="oT")
                        njt = 8 * qc + 8
                        rec = None

                        def do_av(jt, et, regs):
                            v0 = jt * VW + h * (DH + 1)
                            for (a, b) in regs:
                                nc.tensor.matmul(
                                    out=oT[:, a:b],
                                    lhsT=vp_all[:, v0:v0 + 128],
                                    rhs=et[:, a:b],
                                    start=(jt == 0),
                                    stop=(jt == (8 * qc + 3 if b <= 512
                                                 else njt - 1)),
                                    skip_group_check=True)

                        # jt-pair bursting: S,S then the previous pair's
                        # AV,AV — fewer stationary switches and
                        # semaphore-gated PE instructions per step
                        pend = []
                        for jt in range(njt):
                            ql0 = max(0, 128 * jt - q0)
                            diag = jt >= 8 * qc
                            regs = []
                            if ql0 < 512:
                                regs.append((ql0, 512))
                            regs.append((max(ql0, 512), 1024))
                            if jt == 6:
                                flush_norm(ps4)
                            ps = ps4.tile([128, 1024], f32, tag="ps",
                                          name="ps")
                            for (a, b) in regs:
                                nc.tensor.matmul(
                                    out=ps[:, a:b],
                                    lhsT=kT[m][hs, jt * 128:
                                               (jt + 1) * 128],
                                    rhs=qT[m][hs, q0 + a:q0 + b],
                                    start=True, stop=True,
                                    skip_group_check=True)
                            et = etp.tile([128, 1024], bf16, tag="et",
                                          name="et")
                            nc.scalar.activation(
                                out=et[:, ql0:1024], in_=ps[:, ql0:1024],
                                func=Exp, scale=0.125)
                            if diag:
                                nc.gpsimd.tensor_mul(
                                    et[:, ql0:ql0 + 128],
                                    et[:, ql0:ql0 + 128], tri01)
                            pend.append((jt, et, regs))
                            if jt % 2 == 1 and len(pend) == 4:
                                do_av(*pend.pop(0))
                                do_av(*pend.pop(0))
                            if jt == 8 * qc + 5:
                                # oT cols 0:512 final (their AV group stopped
                                # at jt 8qc+3): drain the low half here so
                                # the end-of-section DVE chain — which gates
                                # oT reuse (single-buffered) — halves
                                if hh == 0:
                                    nc.vector.tensor_copy(
                                        aT[m][0:64, q0:q0 + 512],
                                        oT[0:64, 0:512])
                                    dtm = dtp.tile([65, 1024], f32,
                                                   tag="dtm", name="dtm")
                                    nc.vector.tensor_copy(dtm[64:65, 0:512],
                                                          oT[64:65, 0:512])
                                    nc.sync.dma_start(
                                        out=den[m][0:1, q0:q0 + 512],
                                        in_=dtm[64:65, 0:512])
                                else:
                                    nc.vector.tensor_copy(
                                        otmp[0:64, q0:q0 + 512],
                                        oT[0:64, 0:512])
                                    nc.vector.tensor_copy(
                                        den[m][64:65, q0:q0 + 512],
                                        oT[64:65, 0:512])
                                    rec = dtp.tile([65, 1024], f32,
                                                   tag="dtm", name="rec")
                                    nc.vector.reciprocal_approx_fast(
                                        out=rec[:, 0:512],
                                        in_=den[m][:, q0:q0 + 512])
                                    nc.vector.tensor_copy(
                                        recb[0:65, q0:q0 + 512],
                                        rec[:, 0:512])
                            if filler is not None:
                                for _ in range(pops):
                                    next(filler, None)
                        for p_ in pend:
                            do_av(*p_)
                        # drain oT high half: rows 0..63 -> aT / otmp,
                        # row 64 -> den
                        if hh == 0:
                            nc.vector.tensor_copy(
                                aT[m][0:64, q0 + 512:q0 + 1024],
                                oT[0:64, 512:1024])
                            dtm = dtp.tile([65, 1024], f32, tag="dtm",
                                           name="dtm")
                            nc.vector.tensor_copy(dtm[64:65, 512:1024],
                                                  oT[64:65, 512:1024])
                            nc.sync.dma_start(
                                out=den[m][0:1, q0 + 512:q0 + 1024],
                                in_=dtm[64:65, 512:1024])
                        else:
                            nc.vector.tensor_copy(
                                otmp[0:64, q0 + 512:q0 + 1024],
                                oT[0:64, 512:1024])
                            nc.vector.tensor_copy(
                                den[m][64:65, q0 + 512:q0 + 1024],
                                oT[64:65, 512:1024])
                            # pair half complete: repartition odd head,
                            # then normalize this query-half
                            nc.sync.dma_start(
                                out=aT[m][64:128, q0:q0 + 1024],
                                in_=otmp[:, q0:q0 + 1024])
                            nc.vector.reciprocal_approx_fast(
                                out=rec[:, 512:1024],
                                in_=den[m][:, q0 + 512:q0 + 1024])
                            nc.vector.tensor_copy(
                                recb[0:65, q0 + 512:q0 + 1024],
                                rec[:, 512:1024])
                            pnorm.append((m, q0))

            for m in range(NPAIR):
                gens = []
                fmap = {}
                if m + 1 < NPAIR:
                    g = qkproj_gen(m + 1)
                else:
                    g = yp_gen()
                gens.append(g)
                for sec in ((0, 1), (0, 0), (1, 0), (1, 1)):
                    fmap[sec] = (g, 1)
                if m == 0:
                    gv = vproj_gen(12, 16)
                    gens.append(gv)
                    fmap[(0, 1)] = (gv, 2)
                if m + 1 == NPAIR:
                    # drain the output-projection partials fully in-section
                    # instead of as a serial lump before the tail
                    fmap[(1, 1)] = (g, 2)
                emit_pair(m, fmap)
                for g in gens:
                    for _ in g:
                        pass

            # ---------------- output projection tail: f=2 + oacc ---------
            for st in range(16):
                if st == 4:
                    # last query-half normalize: emitted here so st 0..3's
                    # matmuls cover the DVE den->rec chain; only st>=8 reads
                    # the columns this mul produces
                    flush_norm(ps4)
                # alternate ps4/pj tiles: 4 effective PSUM buffers hide the
                # matmul->cast->reuse round trip that a 2-buffer rotation
                # exposes (~0.5us/st)
                if st % 2 == 0:
                    big = ps4.tile([128, 1024], f32, tag="ps", name="ps")
                    ps, pk_view = big[:, 0:512], big[:, 512:768]
                else:
                    psq = pj.tile([128, 512], f32, tag="pjq", name="pjq")
                    psk = pj.tile([128, 512], f32, tag="pjk", name="pjk")
                    ps, pk_view = psq, psk[:, 0:256]
                for (view, a) in ((ps, 0), (pk_view, 512)):
                    w = 768 - a if a else 512
                    nc.tensor.matmul(
                        out=view[:],
                        lhsT=aTb[2][:, st * 128:(st + 1) * 128],
                        rhs=wos[:, 2 * D + a:2 * D + a + w],
                        start=True, stop=False, skip_group_check=True)
                    # += oacc via identity weights: the PE has tail slack,
                    # keeping the combine off the DVE
                    nc.tensor.matmul(
                        out=view[:], lhsT=ident,
                        rhs=oacc[:, st * D + a:st * D + a + w],
                        start=False, stop=True, skip_group_check=True)
                ot = osb.tile([128, D], bf16, tag="ot", name="ot")
                nc.vector.tensor_copy(ot[:, 0:512], ps[:])
                nc.scalar.copy(ot[:, 512:768], pk_view[:])
                # alternate output DMAs across both hardware queues
                qeng = nc.sync if st % 2 == 0 else nc.scalar
                qeng.dma_start(out=o[st * 128:(st + 1) * 128, :],
                               in_=ot[:])

    nc.finalize()
    return nc


def _mask_tiles():
    import ml_dtypes
    # tri01[jp, q] = 1 iff key jp <= query q (within the diagonal tile)
    tri01 = np.triu(np.ones((128, 128), np.float32))
    # sel broadcasts rec rows (0 -> out 0..63, 64 -> out 64..127)
    sel = np.zeros((128, 128), np.float32)
    sel[0, 0:64] = 1.0
    sel[64, 64:128] = 1.0
    # identity: PE-side "+= oacc" in the output projection tail
    ident = np.eye(128, dtype=np.float32)
    return np.stack([tri01, sel, ident]).astype(ml_dtypes.bfloat16)


def _host_reference(x, mask, w_qkv, w_out):
    qkv = x.astype(np.float64) @ w_qkv.astype(np.float64)
    q, k, v = np.split(qkv, 3, axis=-1)

    def heads(t):
        return t.reshape(B, N, H, DH).transpose(0, 2, 1, 3)
    q, k, v = heads(q), heads(k), heads(v)
    s = np.einsum('bhqd,bhkd->bhqk', q, k) / np.sqrt(DH)
    s = np.where(np.asarray(mask).reshape(1, 1, N, N) == 0, -np.inf, s)
    s = s - s.max(-1, keepdims=True)
    e = np.exp(s)
    p = e / e.sum(-1, keepdims=True)
    out = np.einsum('bhqk,bhkd->bhqd', p, v)
    out = out.transpose(0, 2, 1, 3).reshape(B, N, D)
    return (out @ w_out.astype(np.float64)).astype(np.float32)


def kernel(x, mask, w_qkv, w_out):
    import ml_dtypes
    bf = ml_dtypes.bfloat16
    x = np.asarray(x)
    w_qkv = np.asarray(w_qkv)
    w_out = np.asarray(w_out)

    causal = np.array_equal(
        np.asarray(mask).reshape(N, N) != 0, np.tril(np.ones((N, N), bool)))
    if not causal:
        return _host_reference(x, mask, w_qkv, w_out)

    from concourse.bass_utils import run_bass_kernel_spmd
    if "nc" not in _CACHE:
        _CACHE["nc"] = _build_nc()
    nc = _CACHE["nc"]

    cstn = _mask_tiles()
    W = HH * DH  # 384
    wqk_h, wv_h, wo_h = [], [], []
    for hg in range(2):
        wqk_h.append(np.ascontiguousarray(np.concatenate(
            [w_qkv[:, hg * W:(hg + 1) * W],
             w_qkv[:, D + hg * W:D + (hg + 1) * W]], axis=1)).astype(bf))
        wv_h.append(np.ascontiguousarray(
            w_qkv[:, 2 * D + hg * W:2 * D + (hg + 1) * W]).astype(bf))
        wo_h.append(np.ascontiguousarray(
            w_out[hg * W:(hg + 1) * W, :]).astype(bf))
    xts = [np.ascontiguousarray(x[b].T).astype(bf) for b in range(B)]

    in_maps = []
    for c in range(8):
        b, hg = c // 2, c % 2
        in_maps.append({
            "xt": xts[b],
            "wqk": wqk_h[hg], "wv": wv_h[hg], "wo": wo_h[hg],
            "cst": cstn,
        })
    res = run_bass_kernel_spmd(nc, in_maps, core_ids=list(range(8)),
                               **_CACHE.get("run_kwargs", {}))
    _CACHE["last_res"] = res
    out = np.empty((B, N, D), np.float32)
    for b in range(B):
        out[b] = (res.results[2 * b]["o"].astype(np.float32)
                  + res.results[2 * b + 1]["o"].astype(np.float32))
    return out
